# revision 3
# baseline (speedup 1.0000x reference)
"""Bass/Tile TRN2 kernel for nn_MultiHeadSelfAttention (B=2, S=2048, D=1024, H=16).

Sharding: 8 cores; core c handles batch b=c//4 and 4 heads hg=c%4 (e-slice of 256).
Per-core device program (SPMD, same NEFF, different data):
  - Q/K projections into transposed layout qT/kT [e, s] (f32r matmuls),
    V projection into natural layout [s, e] with an appended ones column.
  - Per head: scoresT[k,q] = kT_h^T @ qT_h (contraction d=64), exp on ACT
    (mask folded in as a per-partition bias) -> expT bf16,
    ctx~T[65, q] = vv_h^T @ expT accumulated over k in PSUM; row 64 = Z.
  - 1/Z via DVE reciprocal, broadcast across partitions on GpSimd.
  - attn partial: acc[k,q] += expT_h * (1/Z_h) on DVE (bf16).
  - out-proj: outT[f, s] partial = Wo_slice^T-chunks @ ctxT (f32r).
Host: gathers/sums the 4 per-batch core partials, transposes, adds biases.
"""

import os
import sys

sys.path.insert(0, "/opt/trn_rl_repo")

import numpy as np
import concourse.bass as bass  # noqa: F401  (import order matters for mybir)
import concourse.mybir as mybir
import concourse.tile as tile
from concourse import bacc
from concourse.bass_utils import run_bass_kernel_spmd

DT = mybir.dt
AF = mybir.ActivationFunctionType
OP = mybir.AluOpType

B, S, D, H, HD = 2, 2048, 1024, 16, 64
NCORES = 8
HPC = 4            # heads per core
ESL = HPC * HD     # 256, e-slice width per core
QH = 2             # q halves
QW = S // QH       # 1024
NKT = S // 128     # 16 k-tiles
NDC = D // 128     # 8 d-chunks

_cache = {}


def _build():
    if "nc" in _cache:
        return _cache["nc"], _cache["names"]
    nc = bacc.Bacc(None, target_bir_lowering=False)

    xT_d = nc.dram_tensor("xT", [D, S], DT.float32r, kind="ExternalInput")
    wq_d = nc.dram_tensor("wq", [D, ESL], DT.float32r, kind="ExternalInput")
    wk_d = nc.dram_tensor("wk", [D, ESL], DT.float32r, kind="ExternalInput")
    wv_d = nc.dram_tensor("wv", [D, ESL], DT.float32r, kind="ExternalInput")
    wo_d = nc.dram_tensor("wo", [ESL, D], DT.float32r, kind="ExternalInput")
    bq_d = nc.dram_tensor("bq", [128, 2], DT.float32, kind="ExternalInput")
    bk_d = nc.dram_tensor("bk", [128, 2], DT.float32, kind="ExternalInput")
    mb_d = nc.dram_tensor("mb", [128, NKT], DT.float32, kind="ExternalInput")
    accT_d = nc.dram_tensor("accT", [S, S], DT.bfloat16, kind="ExternalOutput")
    outT_d = nc.dram_tensor("outT", [D, S], DT.float32, kind="ExternalOutput")

    with tile.TileContext(nc) as tc:
        with tc.tile_pool(name="persist", bufs=1) as persist:
            qTs = persist.tile([128, 2, S], DT.float32r)
            kTs = persist.tile([128, 2, S], DT.float32r)
            vv = persist.tile([128, HPC, NKT, 68], DT.bfloat16)   # [s%128, h, kt, hd|1|pad]
            wos = persist.tile([128, 2, D], DT.float32r)
            ctxT2 = persist.tile([128, 2, S], DT.float32r)
            bq_t = persist.tile([128, 2], DT.float32)
            bk_t = persist.tile([128, 2], DT.float32)
            mb_t = persist.tile([128, NKT], DT.float32)

            nc.sync.dma_start(out=wos[:], in_=wo_d[:].rearrange("(c p) f -> p c f", p=128))
            nc.sync.dma_start(out=bq_t[:], in_=bq_d[:])
            nc.sync.dma_start(out=bk_t[:], in_=bk_d[:])
            nc.sync.dma_start(out=mb_t[:], in_=mb_d[:])
            nc.vector.memset(vv[:], 1.0)

            # ---------------- Phase P: projections ----------------
            with tc.tile_pool(name="px", bufs=1) as px, \
                 tc.tile_pool(name="pp", bufs=4, space="PSUM") as pp:
                xTs = px.tile([128, NDC, S], DT.float32r)
                wqs = px.tile([128, NDC, ESL], DT.float32r)
                wks = px.tile([128, NDC, ESL], DT.float32r)
                wvs = px.tile([128, NDC, ESL], DT.float32r)
                nc.sync.dma_start(out=xTs[:], in_=xT_d[:].rearrange("(c p) s -> p c s", p=128))
                nc.sync.dma_start(out=wqs[:], in_=wq_d[:].rearrange("(c p) e -> p c e", p=128))
                nc.sync.dma_start(out=wks[:], in_=wk_d[:].rearrange("(c p) e -> p c e", p=128))
                nc.sync.dma_start(out=wvs[:], in_=wv_d[:].rearrange("(c p) e -> p c e", p=128))

                # q/k projections -> transposed [e, s] layout
                for wsrc, bias_t, dst in ((wqs, bq_t, qTs), (wks, bk_t, kTs)):
                    for ec in range(2):
                        for sb in range(4):
                            ps = pp.tile([128, 512], DT.float32, tag="ppqk")
                            for dc in range(NDC):
                                nc.tensor.matmul(
                                    ps[:],
                                    lhsT=wsrc[:, dc, ec * 128:(ec + 1) * 128],
                                    rhs=xTs[:, dc, sb * 512:(sb + 1) * 512],
                                    start=(dc == 0), stop=(dc == NDC - 1),
                                )
                            nc.vector.tensor_scalar(
                                out=dst[:, ec, sb * 512:(sb + 1) * 512],
                                in0=ps[:], scalar1=bias_t[:, ec:ec + 1], scalar2=None,
                                op0=OP.add,
                            )

                # v projection -> natural [s, e] layout, per-head slices of vv
                for sc in range(NKT):
                    ps = pp.tile([128, ESL], DT.float32, tag="ppv")
                    for dc in range(NDC):
                        nc.tensor.matmul(
                            ps[:],
                            lhsT=xTs[:, dc, sc * 128:(sc + 1) * 128],
                            rhs=wvs[:, dc, :],
                            start=(dc == 0), stop=(dc == NDC - 1),
                        )
                    for h in range(HPC):
                        nc.scalar.activation(
                            vv[:, h, sc, 0:HD], ps[:, h * HD:(h + 1) * HD], AF.Copy,
                        )

            # ---------------- Phase A: attention ----------------
            with tc.tile_pool(name="pa_exp", bufs=2) as pa_exp, \
                 tc.tile_pool(name="pa_acc", bufs=1) as pa_acc, \
                 tc.tile_pool(name="pa_sm", bufs=2) as pa_sm, \
                 tc.tile_pool(name="ps_sc", bufs=2, space="PSUM") as ps_sc, \
                 tc.tile_pool(name="ps_ctx", bufs=2, space="PSUM") as ps_ctx:
                for qh in range(QH):
                    acc_t = pa_acc.tile([128, NKT, QW], DT.bfloat16, tag="acc")
                    for h in range(HPC):
                        hc, hp = h // 2, (h % 2) * 64
                        expT_t = pa_exp.tile([128, NKT, QW], DT.bfloat16, tag="expT")
                        ctxp = ps_ctx.tile([65, QW], DT.float32, tag="ctxp")
                        for kt in range(NKT):
                            scp = ps_sc.tile([128, QW], DT.float32, tag="scp")
                            for qq in range(QW // 512):
                                nc.tensor.matmul(
                                    scp[:, qq * 512:(qq + 1) * 512],
                                    lhsT=kTs[hp:hp + 64, hc, kt * 128:(kt + 1) * 128],
                                    rhs=qTs[hp:hp + 64, hc,
                                            qh * QW + qq * 512: qh * QW + (qq + 1) * 512],
                                    start=True, stop=True,
                                )
                            nc.scalar.activation(
                                expT_t[:, kt, :], scp[:], AF.Exp,
                                bias=mb_t[:, kt:kt + 1],
                            )
                            for qq in range(QW // 512):
                                nc.tensor.matmul(
                                    ctxp[:, qq * 512:(qq + 1) * 512],
                                    lhsT=vv[:, h, kt, 0:65],
                                    rhs=expT_t[:, kt, qq * 512:(qq + 1) * 512],
                                    start=(kt == 0), stop=(kt == NKT - 1),
                                )
                        # Z -> 1/Z -> broadcast tiles
                        zi = pa_sm.tile([1, QW], DT.float32, tag="zi")
                        nc.vector.reciprocal(zi[:], ctxp[64:65, :])
                        zib = pa_sm.tile([1, QW], DT.bfloat16, tag="zib")
                        nc.vector.tensor_copy(zib[:], zi[:])
                        zbf = pa_sm.tile([128, QW], DT.float32, tag="zbf")
                        nc.gpsimd.partition_broadcast(zbf[:], zi[:])
                        zbb = pa_sm.tile([128, QW], DT.bfloat16, tag="zbb")
                        nc.gpsimd.partition_broadcast(zbb[:], zib[:])
                        # normalized ctxT slice (f32r) for the out-projection
                        nc.vector.tensor_tensor(
                            out=ctxT2[hp:hp + 64, hc, qh * QW:(qh + 1) * QW],
                            in0=ctxp[0:64, :], in1=zbf[0:64, :], op=OP.mult,
                        )
                        # attention-mean partial: acc += expT * (1/Z)
                        for kt in range(NKT):
                            if h == 0:
                                nc.vector.tensor_tensor(
                                    out=acc_t[:, kt, :], in0=expT_t[:, kt, :],
                                    in1=zbb[:], op=OP.mult,
                                )
                            else:
                                tmp = pa_sm.tile([128, QW], DT.bfloat16, tag="tmp")
                                nc.vector.tensor_tensor(
                                    out=tmp[:], in0=expT_t[:, kt, :], in1=zbb[:],
                                    op=OP.mult,
                                )
                                nc.vector.tensor_tensor(
                                    out=acc_t[:, kt, :], in0=acc_t[:, kt, :],
                                    in1=tmp[:], op=OP.add,
                                )
                    for kt in range(NKT):
                        nc.sync.dma_start(
                            out=accT_d[kt * 128:(kt + 1) * 128, qh * QW:(qh + 1) * QW],
                            in_=acc_t[:, kt, :],
                        )

            # ---------------- Phase O: out projection ----------------
            with tc.tile_pool(name="po_out", bufs=4) as po_out, \
                 tc.tile_pool(name="ps_o", bufs=4, space="PSUM") as ps_o:
                for ft in range(D // 128):
                    pos = [ps_o.tile([128, 512], DT.float32, tag="pso", name=f"pso{i}")
                           for i in range(4)]
                    for ec in range(2):
                        for sb in range(4):
                            nc.tensor.matmul(
                                pos[sb][:],
                                lhsT=wos[:, ec, ft * 128:(ft + 1) * 128],
                                rhs=ctxT2[:, ec, sb * 512:(sb + 1) * 512],
                                start=(ec == 0), stop=(ec == 1),
                            )
                    for sb in range(4):
                        ot = po_out.tile([128, 512], DT.float32, tag="ot")
                        nc.scalar.activation(ot[:], pos[sb][:], AF.Copy)
                        nc.sync.dma_start(
                            out=outT_d[ft * 128:(ft + 1) * 128, sb * 512:(sb + 1) * 512],
                            in_=ot[:],
                        )

    nc.compile()
    names = dict(acc="accT", out="outT")
    _cache["nc"] = nc
    _cache["names"] = names
    return nc, names


def _prep_inputs(x, mask, Wq, bq, Wk, bk, Wv, bv, Wo, bo):
    """Build the 8 per-core input maps (host-side shard + transpose)."""
    x = np.asarray(x, np.float32)
    mask = np.asarray(mask)
    Wq = np.asarray(Wq, np.float32); bq = np.asarray(bq, np.float32)
    Wk = np.asarray(Wk, np.float32); bk = np.asarray(bk, np.float32)
    Wv = np.asarray(Wv, np.float32)
    Wo = np.asarray(Wo, np.float32)

    WqT = np.ascontiguousarray(Wq.T) / 8.0   # scores scale folded in
    WkT = np.ascontiguousarray(Wk.T)
    WvT = np.ascontiguousarray(Wv.T)
    WoT = np.ascontiguousarray(Wo.T)

    in_maps = []
    for c in range(NCORES):
        b, hg = c // HPC, c % HPC
        esl = slice(hg * ESL, (hg + 1) * ESL)
        mb = (-1e9 * (1.0 - mask[b].astype(np.float32)))
        in_maps.append({
            "xT": np.ascontiguousarray(x[b].T),
            "wq": np.ascontiguousarray(WqT[:, esl]),
            "wk": np.ascontiguousarray(WkT[:, esl]),
            "wv": np.ascontiguousarray(WvT[:, esl]),
            "wo": np.ascontiguousarray(WoT[esl, :]),
            "bq": np.ascontiguousarray((bq[esl] / 8.0).reshape(2, 128).T),
            "bk": np.ascontiguousarray(bk[esl].reshape(2, 128).T),
            "mb": np.ascontiguousarray(mb.reshape(NKT, 128).T),
        })
    return in_maps


def _run(inputs, trace=False):
    nc, _ = _build()
    in_maps = _prep_inputs(**inputs)
    res = run_bass_kernel_spmd(nc, in_maps, core_ids=list(range(NCORES)),
                               trace=trace)
    bv = np.asarray(inputs["bv"], np.float32)
    bo = np.asarray(inputs["bo"], np.float32)
    Wo = np.asarray(inputs["Wo"], np.float32)
    corr = bv @ Wo.T + bo   # sum_k p = 1 makes bv a constant additive term

    out = np.empty((B, S, D), np.float32)
    attn = np.empty((B, S, S), np.float32)
    for b in range(B):
        outT = np.zeros((D, S), np.float32)
        accT = np.zeros((S, S), np.float32)
        for hg in range(HPC):
            r = res.results[b * HPC + hg]
            outT += r["outT"]
            accT += r["accT"].astype(np.float32)
        out[b] = outT.T + corr
        attn[b] = accT.T / float(H)
    return (out, attn), res


def kernel(**inputs):
    (out, attn), _ = _run(inputs, trace=False)
    return out, attn
